# revision 36
# baseline (speedup 1.0000x reference)
"""CQAttention Trainium2 kernel.

Math (per batch b, H=256, q=2048, d=8192):
  Qp   = gelu(Q @ W.T + b)                       [q, H]
  S    = C @ Qp.T                                [d, q]
  P    = softmax(S, axis=q)
  out  = P @ Qp + C                              [d, H]

Sharding: data-parallel over batch, one batch per NeuronCore (8 cores).

Per-core pipeline (all matmuls contract over the feature dim or q):
  - Q^T, W^T via PE transposes; QpT = gelu(W Q^T + b) with per-partition bias
    on the ACT engine; Qp (natural, bf16) by transposing QpT back, augmented
    with a ones column so the softmax denominator falls out of the second
    matmul's PSUM accumulation.
  - Per 512-row chunk of C: transpose C tiles to put the feature dim on
    partitions; logits^T tiles [q=128, d=512] with fp16 operands (full PE
    rate, ~11-bit mantissa; bf16 fails the 2e-2 gate, fp32r is ~8% slower
    on the moving operand); exp on ACT straight from PSUM to bf16 (softmax
    without max-subtraction: |logits| < ~70 so fp32 exp is safe); attended
    accumulated over the 16 q-tiles into PSUM [d=128, 257] where column 256
    is the row-sum (ones column on Qp); fused epilogue
    out = (attended * 1/rowsum) + C in one DVE op per tile.
  - Chunk pipeline: C loads 3 chunks ahead, C transposes 2 chunks ahead,
    attended lags logits/exp by 2 q-tiles; the Q-side setup is folded into
    chunk 0's step loop so the PE never idles at startup.
"""

from contextlib import ExitStack

import numpy as np

import concourse.mybir as mybir
import concourse.tile as tile
from concourse import bacc
from concourse.bass_utils import run_bass_kernel_spmd
from concourse.masks import make_identity

B, QL, D, H = 8, 2048, 8192, 256
N_CORES = 8
F32 = mybir.dt.float32
F32R = mybir.dt.float32r
BF16 = mybir.dt.bfloat16
F16 = mybir.dt.float16

HC = H // 128      # feature chunks (2)
NQT = QL // 128    # q tiles (16)
DC = 512           # d-chunk size
NDC = D // DC      # d chunks (16)
NDM = DC // 128    # d tiles per chunk (4)

# Dtype of the logits-matmul operands (C^T, QpT, Q^T, W^T).
# f32r: full fp32 storage, PE float32r mode. f16: half storage, full PE rate,
# ~11-bit mantissa (logit noise ~0.006 abs vs softmax scale ~10). bf16: fails
# the 2e-2 gate (2.7e-2) -- do not use.
LOGITS_DT = "f16"

AF = mybir.ActivationFunctionType
ALU = mybir.AluOpType


LS = {"f32r": F32R, "bf16": BF16, "f16": F16}[LOGITS_DT]


def build_body(ctx: ExitStack, tc: tile.TileContext, nc, Qd, Cd, Wd, bd, Od):
    singles = ctx.enter_context(tc.tile_pool(name="singles", bufs=1))
    qstat = ctx.enter_context(tc.tile_pool(name="qstat", bufs=1))
    cpool = ctx.enter_context(tc.tile_pool(name="cpool", bufs=5))
    ctpool = ctx.enter_context(tc.tile_pool(name="ctp", bufs=3))
    exppool = ctx.enter_context(tc.tile_pool(name="expp", bufs=2))
    outpool = ctx.enter_context(tc.tile_pool(name="outp", bufs=3))
    small = ctx.enter_context(tc.tile_pool(name="small", bufs=4))
    psum_l = ctx.enter_context(tc.tile_pool(name="psl", bufs=2, space="PSUM"))
    psum_t = ctx.enter_context(tc.tile_pool(name="pst", bufs=2, space="PSUM"))
    psum_a = ctx.enter_context(tc.tile_pool(name="psa", bufs=1, space="PSUM"))

    ident = singles.tile([128, 128], F32 if LOGITS_DT == "f32r" else LS)
    make_identity(nc, ident)
    TDT = F32 if LOGITS_DT == "f32r" else LS

    # --- main loop over d chunks, software-pipelined C prep ---
    def c_load(dc):
        c_nat = cpool.tile([128, NDM, H], F32, tag="cnat", name=f"cnat{dc}")
        nc.sync.dma_start(
            out=c_nat[:],
            in_=Cd[dc * DC:(dc + 1) * DC, :].rearrange("(a p) h -> p a h", p=128))
        if LOGITS_DT == "f32r":
            return c_nat, c_nat
        c_bf = cpool.tile([128, NDM, H], LS, tag="cbf", name=f"cbf{dc}")
        nc.vector.tensor_copy(c_bf[:], c_nat[:])
        return c_nat, c_bf

    def c_transpose(dc, c_nat, hc):
        pt = psum_t.tile([128, 512], TDT, tag="pt", name=f"ptc{dc}_{hc}")
        for dm in range(NDM):
            nc.tensor.transpose(
                pt[:, dm * 128:(dm + 1) * 128],
                c_nat[:, dm, hc * 128:(hc + 1) * 128], ident[:])
        return pt

    c_nats = {}
    cts = {}

    def ct_dma(dc, hc):
        cb = c_nats[dc][1]
        for dm in range(NDM):
            nc.sync.dma_start(
                out=cts[dc][:, hc, dm * 128:(dm + 1) * 128],
                in_=cb[:, dm, hc * 128:(hc + 1) * 128], transpose=True)

    def c_prep(dc):
        cts[dc] = ctpool.tile([128, HC, DC], LS, tag="ct", name=f"ct{dc}")
        for hc in range(HC):
            ct_dma(dc, hc)

    # --- W^T [h, o] and bias ---
    w_nat = singles.tile([128, HC, H], F32)  # [o in-chunk, om, h]
    nc.sync.dma_start(out=w_nat[:],
                      in_=Wd.rearrange("(a p) h -> p a h", p=128))
    if LOGITS_DT == "f32r":
        w_src = w_nat
    else:
        w_src = singles.tile([128, HC, H], LS, name="w_bf")
        for om in range(HC):
            nc.vector.tensor_copy(w_src[:, om, :], w_nat[:, om, :])
    wt = qstat.tile([128, HC, H], LS)  # [h in-chunk, hc, o]
    for om in range(HC):
        for hc in range(HC):
            pt = psum_t.tile([128, 128], TDT)
            nc.tensor.transpose(pt[:], w_src[:, om, hc * 128:(hc + 1) * 128], ident[:])
            nc.vector.tensor_copy(wt[:, hc, om * 128:(om + 1) * 128], pt[:])
    bias = singles.tile([128, HC, 1], F32)
    nc.sync.dma_start(out=bias[:, :, 0], in_=bd.rearrange("(c p) -> p c", p=128))
    c_nats[0] = c_load(0)
    c_prep(0)

    # --- per-group pipeline: Q^T -> linear+gelu -> Qp ---
    qt = qstat.tile([128, HC, QL], LS)
    qpt = qstat.tile([128, HC, QL], LS)
    qp = qstat.tile([128, NQT, H + 1], BF16)
    q_nat = cpool.tile([128, NQT, H], F32, tag="qnat", bufs=1)
    q_view = Qd.rearrange("(a p) h -> p a h", p=128)
    if LOGITS_DT == "f32r":
        q_src = q_nat
    else:
        q_src = cpool.tile([128, NQT, H], LS, tag="qbf", bufs=1)
    for qg in range(NQT // 2):
        nc.sync.dma_start(out=q_nat[:, qg * 2:(qg + 1) * 2, :],
                          in_=q_view[:, qg * 2:(qg + 1) * 2, :])
        if LOGITS_DT != "f32r":
            nc.vector.tensor_copy(q_src[:, qg * 2:(qg + 1) * 2, :],
                                  q_nat[:, qg * 2:(qg + 1) * 2, :])
    for _dc in range(1, 4):
        c_nats[_dc] = c_load(_dc)
    _setup_pt = [0]
    def setup_pt_tile(name):
        k = _setup_pt[0] % 6
        _setup_pt[0] += 1
        if k < 4:
            return psum_a.tile([128, 512], TDT, tag=f"a{k}", name=name)
        return psum_t.tile([128, 512], TDT, tag="pt", name=name)

    def q_group(qg, in_loop):
        def pt_tile(name):
            if in_loop:
                return psum_t.tile([128, 512], TDT, tag="pt", name=name)
            return setup_pt_tile(name)
        # Q^T for this group of 4 q-tiles
        for hc in range(HC):
            pt = pt_tile(f"ptq{qg}_{hc}")
            for k in range(4):
                qi = qg * 4 + k
                nc.tensor.transpose(
                    pt[:, k * 128:(k + 1) * 128],
                    q_src[:, qi, hc * 128:(hc + 1) * 128], ident[:])
            nc.vector.tensor_copy(qt[:, hc, qg * 512:(qg + 1) * 512], pt[:])
        # linear + gelu for this 512-wide q block
        for om in range(HC):
            pl = psum_l.tile([128, 512], F32, tag="pl", name=f"plin{qg}_{om}")
            for hc in range(HC):
                nc.tensor.matmul(
                    pl[:],
                    wt[:, hc, om * 128:(om + 1) * 128],
                    qt[:, hc, qg * 512:(qg + 1) * 512],
                    start=(hc == 0),
                    stop=(hc == HC - 1),
                )
            nc.scalar.activation(
                qpt[:, om, qg * 512:(qg + 1) * 512], pl[:], AF.Gelu,
                bias=bias[:, om, :], scale=1.0,
            )
        # Qp natural for this group
        for om in range(HC):
            pt = pt_tile(f"ptp{qg}_{om}")
            for k in range(4):
                qi = qg * 4 + k
                s = qpt[:, om, qi * 128:(qi + 1) * 128]
                if LOGITS_DT == "f32r":
                    s = s.bitcast(F32)
                nc.tensor.transpose(pt[:, k * 128:(k + 1) * 128], s, ident[:])
            nc.vector.tensor_copy(
                qp[:, qg * 4:(qg + 1) * 4, om * 128:(om + 1) * 128],
                pt.rearrange("p (a b) -> p a b", a=4))
    q_group(0, False)
    c_prep(1)
    nc.vector.memset(qp[:, :, H:H + 1], 1.0)

    # Lag the attended matmuls two q-tiles behind logits+exp so the PE
    # never waits on the ACT exp latency.
    LAG = 2
    for dc in range(NDC):
        c_nat = c_nats[dc][0]
        ct = cts[dc]
        expt = exppool.tile([128, NQT, DC], BF16)
        pa = [psum_a.tile([128, H + 1], F32, tag=f"a{dm}", name=f"pa{dm}")
              for dm in range(NDM)]
        nxt = dc + 2
        tsteps = (15, 17) if dc == 0 else (8, 11)
        for step in range(NQT + LAG):
            if dc == 0 and step in (4, 8, 12):
                q_group(step // 4, True)
            if step == 2 and dc + 3 < NDC:
                c_nats[dc + 3] = c_load(dc + 3)
            if step == 6 and nxt < NDC and nxt not in cts:
                cts[nxt] = ctpool.tile([128, HC, DC], LS, tag="ct", name=f"ct{nxt}")
            if step in tsteps and nxt < NDC and nxt > 1:
                hc = 0 if step == tsteps[0] else 1
                ct_dma(nxt, hc)
            if step < NQT:
                qi = step
                if qi in (5, 13) and dc > 0:
                    pl = psum_t.tile([128, DC], F32, tag="pt", name=f"plx{dc}_{qi}")
                else:
                    pl = psum_l.tile([128, DC], F32)
                for hc in range(HC):
                    nc.tensor.matmul(
                        pl[:],
                        qpt[:, hc, qi * 128:(qi + 1) * 128],
                        ct[:, hc, :],
                        start=(hc == 0),
                        stop=(hc == HC - 1),
                    )
                nc.scalar.activation(expt[:, qi, :], pl[:], AF.Exp)
            if step >= LAG:
                qj = step - LAG
                for dm in range(NDM):
                    nc.tensor.matmul(
                        pa[dm][:],
                        expt[:, qj, dm * 128:(dm + 1) * 128],
                        qp[:, qj, :],
                        start=(qj == 0),
                        stop=(qj == NQT - 1),
                    )

        o_sb = outpool.tile([128, NDM, H], F32)
        for dm in range(NDM):
            rec = small.tile([128, 1], F32)
            nc.vector.reciprocal(rec[:], pa[dm][:, H:H + 1])
            nc.vector.scalar_tensor_tensor(
                o_sb[:, dm, :], pa[dm][:, 0:H], rec[:], c_nat[:, dm, :],
                ALU.mult, ALU.add,
            )
        nc.sync.dma_start(
            out=Od[dc * DC:(dc + 1) * DC, :].rearrange("(a p) h -> p a h", p=128),
            in_=o_sb[:])
        del c_nats[dc], cts[dc]


def build_nc():
    nc = bacc.Bacc("TRN2", target_bir_lowering=False, debug=False,
                   num_devices=N_CORES)
    Qd = nc.dram_tensor("Q", [QL, H], F32, kind="ExternalInput")
    Cd = nc.dram_tensor("C", [D, H], F32, kind="ExternalInput")
    Wd = nc.dram_tensor("W", [H, H], F32, kind="ExternalInput")
    bd = nc.dram_tensor("b", [H], F32, kind="ExternalInput")
    Od = nc.dram_tensor("out", [D, H], F32, kind="ExternalOutput")
    with tile.TileContext(nc) as tc:
        with ExitStack() as ctx:
            build_body(ctx, tc, nc, Qd[:], Cd[:], Wd[:], bd[:], Od[:])
    nc.finalize()
    return nc


_NC = None


def get_nc():
    global _NC
    if _NC is None:
        _NC = build_nc()
    return _NC


def kernel(Q, C, W, b):
    assert Q.shape == (B, QL, H) and C.shape == (B, D, H)
    nc = get_nc()
    in_maps = [
        {
            "Q": np.ascontiguousarray(Q[i], dtype=np.float32),
            "C": np.ascontiguousarray(C[i], dtype=np.float32),
            "W": np.ascontiguousarray(W, dtype=np.float32),
            "b": np.ascontiguousarray(b, dtype=np.float32),
        }
        for i in range(N_CORES)
    ]
    res = run_bass_kernel_spmd(nc, in_maps, core_ids=list(range(N_CORES)))
    return np.stack([res.results[i]["out"] for i in range(N_CORES)], axis=0)


# revision 37
# speedup vs baseline: 1.2726x; 1.2726x over previous
"""CQAttention Trainium2 kernel.

Math (per batch b, H=256, q=2048, d=8192):
  Qp   = gelu(Q @ W.T + b)                       [q, H]
  S    = C @ Qp.T                                [d, q]
  P    = softmax(S, axis=q)
  out  = P @ Qp + C                              [d, H]

Sharding: data-parallel over batch, one batch per NeuronCore (8 cores).

Per-core pipeline (all matmuls contract over the feature dim or q):
  - Q^T, W^T via PE transposes; QpT = gelu(W Q^T + b) with per-partition bias
    on the ACT engine; Qp (natural, bf16) by transposing QpT back, augmented
    with a ones column so the softmax denominator falls out of the second
    matmul's PSUM accumulation.
  - Per 512-row chunk of C: transpose C tiles to put the feature dim on
    partitions; logits^T tiles [q=128, d=512] with fp16 operands (full PE
    rate, ~11-bit mantissa; bf16 fails the 2e-2 gate, fp32r is ~8% slower
    on the moving operand); exp on ACT straight from PSUM to bf16 (softmax
    without max-subtraction: |logits| < ~70 so fp32 exp is safe); attended
    accumulated over the 16 q-tiles into PSUM [d=128, 257] where column 256
    is the row-sum (ones column on Qp); fused epilogue
    out = (attended * 1/rowsum) + C in one DVE op per tile.
  - Chunk pipeline: C loads 3 chunks ahead, C transposes 2 chunks ahead,
    attended lags logits/exp by 2 q-tiles; the Q-side setup is folded into
    chunk 0's step loop so the PE never idles at startup.
"""

from contextlib import ExitStack

import numpy as np

import concourse.mybir as mybir
import concourse.tile as tile
from concourse import bacc
from concourse.bass_utils import run_bass_kernel_spmd
from concourse.masks import make_identity

B, QL, D, H = 8, 2048, 8192, 256
N_CORES = 8
F32 = mybir.dt.float32
F32R = mybir.dt.float32r
BF16 = mybir.dt.bfloat16
F16 = mybir.dt.float16

HC = H // 128      # feature chunks (2)
NQT = QL // 128    # q tiles (16)
DC = 512           # d-chunk size
NDC = D // DC      # d chunks (16)
NDM = DC // 128    # d tiles per chunk (4)

# Dtype of the logits-matmul operands (C^T, QpT, Q^T, W^T).
# f32r: full fp32 storage, PE float32r mode. f16: half storage, full PE rate,
# ~11-bit mantissa (logit noise ~0.006 abs vs softmax scale ~10). bf16: fails
# the 2e-2 gate (2.7e-2) -- do not use.
LOGITS_DT = "f16"

AF = mybir.ActivationFunctionType
ALU = mybir.AluOpType


LS = {"f32r": F32R, "bf16": BF16, "f16": F16}[LOGITS_DT]


def build_body(ctx: ExitStack, tc: tile.TileContext, nc, Qd, Cd, Wd, bd, Od):
    singles = ctx.enter_context(tc.tile_pool(name="singles", bufs=1))
    qstat = ctx.enter_context(tc.tile_pool(name="qstat", bufs=1))
    cpool = ctx.enter_context(tc.tile_pool(name="cpool", bufs=5))
    ctpool = ctx.enter_context(tc.tile_pool(name="ctp", bufs=3))
    exppool = ctx.enter_context(tc.tile_pool(name="expp", bufs=2))
    outpool = ctx.enter_context(tc.tile_pool(name="outp", bufs=3))
    small = ctx.enter_context(tc.tile_pool(name="small", bufs=4))
    psum_l = ctx.enter_context(tc.tile_pool(name="psl", bufs=2, space="PSUM"))
    psum_t = ctx.enter_context(tc.tile_pool(name="pst", bufs=2, space="PSUM"))
    psum_a = ctx.enter_context(tc.tile_pool(name="psa", bufs=1, space="PSUM"))

    ident = singles.tile([128, 128], F32 if LOGITS_DT == "f32r" else LS)
    make_identity(nc, ident)
    TDT = F32 if LOGITS_DT == "f32r" else LS

    # --- main loop over d chunks, software-pipelined C prep ---
    def c_load(dc):
        c_nat = cpool.tile([128, NDM, H], F32, tag="cnat", name=f"cnat{dc}")
        nc.sync.dma_start(
            out=c_nat[:],
            in_=Cd[dc * DC:(dc + 1) * DC, :].rearrange("(a p) h -> p a h", p=128))
        if LOGITS_DT == "f32r":
            return c_nat, c_nat
        c_bf = cpool.tile([128, NDM, H], LS, tag="cbf", name=f"cbf{dc}")
        nc.vector.tensor_copy(c_bf[:], c_nat[:])
        return c_nat, c_bf

    def c_transpose(dc, c_nat, hc):
        pt = psum_t.tile([128, 512], TDT, tag="pt", name=f"ptc{dc}_{hc}")
        for dm in range(NDM):
            nc.tensor.transpose(
                pt[:, dm * 128:(dm + 1) * 128],
                c_nat[:, dm, hc * 128:(hc + 1) * 128], ident[:])
        return pt

    c_nats = {}
    cts = {}

    def c_prep(dc):
        cts[dc] = ctpool.tile([128, HC, DC], LS, tag="ct", name=f"ct{dc}")
        for hc in range(HC):
            pt = c_transpose(dc, c_nats[dc][1], hc)
            nc.vector.tensor_copy(cts[dc][:, hc, :], pt[:])

    # --- W^T [h, o] and bias ---
    w_nat = singles.tile([128, HC, H], F32)  # [o in-chunk, om, h]
    nc.sync.dma_start(out=w_nat[:],
                      in_=Wd.rearrange("(a p) h -> p a h", p=128))
    if LOGITS_DT == "f32r":
        w_src = w_nat
    else:
        w_src = singles.tile([128, HC, H], LS, name="w_bf")
        for om in range(HC):
            nc.vector.tensor_copy(w_src[:, om, :], w_nat[:, om, :])
    wt = qstat.tile([128, HC, H], LS)  # [h in-chunk, hc, o]
    for om in range(HC):
        for hc in range(HC):
            pt = psum_t.tile([128, 128], TDT)
            nc.tensor.transpose(pt[:], w_src[:, om, hc * 128:(hc + 1) * 128], ident[:])
            nc.vector.tensor_copy(wt[:, hc, om * 128:(om + 1) * 128], pt[:])
    bias = singles.tile([128, HC, 1], F32)
    nc.sync.dma_start(out=bias[:, :, 0], in_=bd.rearrange("(c p) -> p c", p=128))
    c_nats[0] = c_load(0)
    c_prep(0)

    # --- per-group pipeline: Q^T -> linear+gelu -> Qp ---
    qt = qstat.tile([128, HC, QL], LS)
    qpt = qstat.tile([128, HC, QL], LS)
    qp = qstat.tile([128, NQT, H + 1], BF16)
    q_nat = cpool.tile([128, NQT, H], F32, tag="qnat", bufs=1)
    q_view = Qd.rearrange("(a p) h -> p a h", p=128)
    if LOGITS_DT == "f32r":
        q_src = q_nat
    else:
        q_src = cpool.tile([128, NQT, H], LS, tag="qbf", bufs=1)
    for qg in range(NQT // 2):
        nc.sync.dma_start(out=q_nat[:, qg * 2:(qg + 1) * 2, :],
                          in_=q_view[:, qg * 2:(qg + 1) * 2, :])
        if LOGITS_DT != "f32r":
            nc.vector.tensor_copy(q_src[:, qg * 2:(qg + 1) * 2, :],
                                  q_nat[:, qg * 2:(qg + 1) * 2, :])
    for _dc in range(1, 4):
        c_nats[_dc] = c_load(_dc)
    _setup_pt = [0]
    def setup_pt_tile(name):
        k = _setup_pt[0] % 6
        _setup_pt[0] += 1
        if k < 4:
            return psum_a.tile([128, 512], TDT, tag=f"a{k}", name=name)
        return psum_t.tile([128, 512], TDT, tag="pt", name=name)

    def q_group(qg, in_loop):
        def pt_tile(name):
            if in_loop:
                return psum_t.tile([128, 512], TDT, tag="pt", name=name)
            return setup_pt_tile(name)
        # Q^T for this group of 4 q-tiles
        for hc in range(HC):
            pt = pt_tile(f"ptq{qg}_{hc}")
            for k in range(4):
                qi = qg * 4 + k
                nc.tensor.transpose(
                    pt[:, k * 128:(k + 1) * 128],
                    q_src[:, qi, hc * 128:(hc + 1) * 128], ident[:])
            nc.vector.tensor_copy(qt[:, hc, qg * 512:(qg + 1) * 512], pt[:])
        # linear + gelu for this 512-wide q block
        for om in range(HC):
            pl = psum_l.tile([128, 512], F32, tag="pl", name=f"plin{qg}_{om}")
            for hc in range(HC):
                nc.tensor.matmul(
                    pl[:],
                    wt[:, hc, om * 128:(om + 1) * 128],
                    qt[:, hc, qg * 512:(qg + 1) * 512],
                    start=(hc == 0),
                    stop=(hc == HC - 1),
                )
            nc.scalar.activation(
                qpt[:, om, qg * 512:(qg + 1) * 512], pl[:], AF.Gelu,
                bias=bias[:, om, :], scale=1.0,
            )
        # Qp natural for this group
        for om in range(HC):
            pt = pt_tile(f"ptp{qg}_{om}")
            for k in range(4):
                qi = qg * 4 + k
                s = qpt[:, om, qi * 128:(qi + 1) * 128]
                if LOGITS_DT == "f32r":
                    s = s.bitcast(F32)
                nc.tensor.transpose(pt[:, k * 128:(k + 1) * 128], s, ident[:])
            nc.vector.tensor_copy(
                qp[:, qg * 4:(qg + 1) * 4, om * 128:(om + 1) * 128],
                pt.rearrange("p (a b) -> p a b", a=4))
    q_group(0, False)
    c_prep(1)
    nc.vector.memset(qp[:, :, H:H + 1], 1.0)

    # Lag the attended matmuls two q-tiles behind logits+exp so the PE
    # never waits on the ACT exp latency.
    LAG = 2
    for dc in range(NDC):
        c_nat = c_nats[dc][0]
        ct = cts[dc]
        expt = exppool.tile([128, NQT, DC], BF16)
        pa = [psum_a.tile([128, H + 1], F32, tag=f"a{dm}", name=f"pa{dm}")
              for dm in range(NDM)]
        nxt = dc + 2
        tsteps = (15, 17) if dc == 0 else (8, 11)
        for step in range(NQT + LAG):
            if dc == 0 and step in (4, 8, 12):
                q_group(step // 4, True)
            if step == 2 and dc + 3 < NDC:
                c_nats[dc + 3] = c_load(dc + 3)
            if step == 6 and nxt < NDC and nxt not in cts:
                cts[nxt] = ctpool.tile([128, HC, DC], LS, tag="ct", name=f"ct{nxt}")
            if step in tsteps and nxt < NDC and nxt > 1:
                hc = 0 if step == tsteps[0] else 1
                pt = c_transpose(nxt, c_nats[nxt][1], hc)
                nc.vector.tensor_copy(cts[nxt][:, hc, :], pt[:])
            if step < NQT:
                qi = step
                if qi in (5, 13) and dc > 0:
                    pl = psum_t.tile([128, DC], F32, tag="pt", name=f"plx{dc}_{qi}")
                else:
                    pl = psum_l.tile([128, DC], F32)
                for hc in range(HC):
                    nc.tensor.matmul(
                        pl[:],
                        qpt[:, hc, qi * 128:(qi + 1) * 128],
                        ct[:, hc, :],
                        start=(hc == 0),
                        stop=(hc == HC - 1),
                    )
                nc.scalar.activation(expt[:, qi, :], pl[:], AF.Exp)
            if step >= LAG:
                qj = step - LAG
                for dm in range(NDM):
                    nc.tensor.matmul(
                        pa[dm][:],
                        expt[:, qj, dm * 128:(dm + 1) * 128],
                        qp[:, qj, :],
                        start=(qj == 0),
                        stop=(qj == NQT - 1),
                    )

        o_sb = outpool.tile([128, NDM, H], F32)
        for dm in range(NDM):
            rec = small.tile([128, 1], F32)
            nc.vector.reciprocal(rec[:], pa[dm][:, H:H + 1])
            nc.vector.scalar_tensor_tensor(
                o_sb[:, dm, :], pa[dm][:, 0:H], rec[:], c_nat[:, dm, :],
                ALU.mult, ALU.add,
            )
        nc.sync.dma_start(
            out=Od[dc * DC:(dc + 1) * DC, :].rearrange("(a p) h -> p a h", p=128),
            in_=o_sb[:])
        del c_nats[dc], cts[dc]


def build_nc():
    nc = bacc.Bacc("TRN2", target_bir_lowering=False, debug=False,
                   num_devices=N_CORES)
    Qd = nc.dram_tensor("Q", [QL, H], F32, kind="ExternalInput")
    Cd = nc.dram_tensor("C", [D, H], F32, kind="ExternalInput")
    Wd = nc.dram_tensor("W", [H, H], F32, kind="ExternalInput")
    bd = nc.dram_tensor("b", [H], F32, kind="ExternalInput")
    Od = nc.dram_tensor("out", [D, H], F32, kind="ExternalOutput")
    with tile.TileContext(nc) as tc:
        with ExitStack() as ctx:
            build_body(ctx, tc, nc, Qd[:], Cd[:], Wd[:], bd[:], Od[:])
    nc.finalize()
    return nc


_NC = None


def get_nc():
    global _NC
    if _NC is None:
        _NC = build_nc()
    return _NC


def kernel(Q, C, W, b):
    assert Q.shape == (B, QL, H) and C.shape == (B, D, H)
    nc = get_nc()
    in_maps = [
        {
            "Q": np.ascontiguousarray(Q[i], dtype=np.float32),
            "C": np.ascontiguousarray(C[i], dtype=np.float32),
            "W": np.ascontiguousarray(W, dtype=np.float32),
            "b": np.ascontiguousarray(b, dtype=np.float32),
        }
        for i in range(N_CORES)
    ]
    res = run_bass_kernel_spmd(nc, in_maps, core_ids=list(range(N_CORES)))
    return np.stack([res.results[i]["out"] for i in range(N_CORES)], axis=0)
